# revision 8
# baseline (speedup 1.0000x reference)
"""Causal self-attention 2D kernel for Trainium2 (Bass/Tile), data-parallel over batch.

Problem (hardcoded): x (8, 512, 32, 32) f32, W_qkv (1536, 512), b_qkv (1536,),
W_proj (512, 512), b_proj (512,). seq = 32*32 = 1024 in raster order, 8 heads,
head_dim 64, causal softmax.

Sharding: one batch element per NeuronCore (8 cores). Weights broadcast.

Per-core dataflow (bf16 matmul operands, fp32 accumulation):
  - x[b] stored (C, T) = (512, 1024); weights pre-transposed on host.
  - scores computed TRANSPOSED: s^T[key, query] per head pair, both heads'
    128-key blocks in one 2-bank PSUM tile so a single ACT exp covers them.
  - softmax without max-subtraction; causal masking via gpsimd.affine_select
    on diagonal blocks only; fully masked key blocks skipped everywhere.
  - P.V transposed with v stationary; softmax denominator rides as an extra
    ones column of v (even heads: [v | 1] -> u rows 0:64, denom row 64; odd
    heads: [1 | 0*63 | v] -> u rows 64:128, denom row 0).
  - normalize: DVE reciprocal of denom rows -> gpsimd partition_broadcast
    (no PE matmul, no PSUM) -> DVE multiply into (C, T) attention output.
  - single attention stream over the 4 head pairs, scores double-buffered in
    PSUM (2x [128, 2, 512] tiles) so the PE runs 2 steps ahead of ACT exp;
    PV matmuls lag scores by 2 steps; remaining QKV chains and the tg=0
    projection are injected into specific slots as PE filler; the engine mix
    is balanced so ACT (exp) ~ PE in the attention span.
  - PSUM budget: scores ring 4 banks + two PV accumulator rings 4 banks = 8.
    v/qk/proj chain scratch PSUM is allocated from the scores ring in
    parity-preserving (even-count) groups.
"""

import numpy as np

import concourse.bass as bass
import concourse.mybir as mybir
from concourse import bacc
from concourse.tile import TileContext

F32 = mybir.dt.float32
BF16 = mybir.dt.bfloat16
AF = mybir.ActivationFunctionType
ALU = mybir.AluOpType

B, C, H, W = 8, 512, 32, 32
T = H * W            # 1024 tokens
NH, HD = 8, C // 8   # 8 heads, dim 64
P = 128
KT = C // P          # 4 contraction tiles
TG = 512             # token group (matmul free dim)
NTG = T // TG        # 2
TT = T // P          # 8 token tiles (also key tiles)
PAIRS = NH // 2      # 4 head pairs
LAG = 2

_CACHE = {}


def build_nc():
    if "nc" in _CACHE:
        return _CACHE["nc"]
    nc = bacc.Bacc(None, target_bir_lowering=False)

    x = nc.dram_tensor("x", (C, T), BF16, kind="ExternalInput")
    wq = nc.dram_tensor("wq", (C, C), BF16, kind="ExternalInput")  # (c_in, c_out), pre-scaled
    wk = nc.dram_tensor("wk", (C, C), BF16, kind="ExternalInput")
    wv = nc.dram_tensor("wv", (C, C), BF16, kind="ExternalInput")
    wp = nc.dram_tensor("wp", (C, C), BF16, kind="ExternalInput")
    # packed consts: cols 0:128 ones rows (0,64); row 0 cols 128:640 = bv;
    # cols 640:664 hold 12 f32 bias values (bq|bk|bp) as raw bytes
    cpack = nc.dram_tensor("cpack", (P, P + C + 24), BF16, kind="ExternalInput")
    y = nc.dram_tensor("y", (C, T), F32, kind="ExternalOutput")

    x_r = x.rearrange("(ko p) t -> p ko t", p=P)
    w_r = {n: t.rearrange("(ko p) j -> p ko j", p=P) for n, t in
           [("wq", wq), ("wk", wk), ("wv", wv), ("wp", wp)]}

    with TileContext(nc) as tc:
        with (
            tc.tile_pool(name="const", bufs=1) as cpool,
            tc.tile_pool(name="work", bufs=3) as wpool,
            tc.tile_pool(name="apool", bufs=4) as apool,
            tc.tile_pool(name="psS", bufs=2, space="PSUM") as psS,
            tc.tile_pool(name="psO", bufs=2, space="PSUM") as psO,
        ):
            # ---- persistent SBUF ----
            x_sb = [cpool.tile([P, T], BF16, tag=f"x_sb{kc}", name=f"x_sb{kc}") for kc in range(KT)]
            wq_sb = cpool.tile([P, KT, C], BF16, tag="wq_sb")
            wk_sb = cpool.tile([P, KT, C], BF16, tag="wk_sb")
            wv_sb = cpool.tile([P, KT, C], BF16, tag="wv_sb")
            wp_sb = cpool.tile([P, KT, C], BF16, tag="wp_sb")
            cpack_sb = cpool.tile([P, P + C + 24], BF16, tag="cpack_sb")
            bv_bc = cpool.tile([P, C], F32, tag="bv_bc")
            ones_sb = cpack_sb[:, 0:P]
            bv_row = cpack_sb[0:1, P:P + C]
            qT_sb = cpool.tile([P, PAIRS, T], BF16, tag="qT_sb")   # pair m: heads 2m, 2m+1
            kT_sb = cpool.tile([P, PAIRS, T], BF16, tag="kT_sb")
            v_sb = cpool.tile([P, TT, NH, P], BF16, tag="v_sb")  # [t-part, keytile, head, 128]
            at_sb = cpool.tile([P, PAIRS, T], BF16, tag="at_sb")  # attention out (C, T)

            bias_f32 = cpack_sb[:, P + C:P + C + 24].bitcast(F32)  # [P, 12]
            bq_c = bias_f32[:, 0:KT]
            bk_c = bias_f32[:, KT:2 * KT]
            bp_c = bias_f32[:, 2 * KT:3 * KT]

            # ---- DMAs: v-path first (x + wv), then q/k weights, wp last.
            # Spread issue across sequencers (HWDGE is shared; this overlaps
            # the per-engine DGE setup).
            nc.sync.dma_start(cpack_sb[:, :], cpack[:, :])
            nc.sync.dma_start(wv_sb[:, 0, :], w_r["wv"][:, 0, :])
            nc.sync.dma_start(x_sb[0][:], x_r[:, 0, :])
            nc.sync.dma_start(wv_sb[:, 1:KT, :], w_r["wv"][:, 1:KT, :])
            nc.sync.dma_start(x_sb[1][:], x_r[:, 1, :])
            nc.sync.dma_start(x_sb[2][:], x_r[:, 2, :])
            nc.sync.dma_start(x_sb[3][:], x_r[:, 3, :])
            nc.scalar.dma_start(wq_sb[:], w_r["wq"][:, :, :])
            nc.scalar.dma_start(wk_sb[:], w_r["wk"][:, :, :])
            nc.scalar.dma_start(wp_sb[:], w_r["wp"][:, :, :])

            # ---- gpsimd constants: v_aug columns; bv broadcast ----
            # even heads col 64 = 1; odd heads col 0 = 1, cols 1:64 = 0
            nc.gpsimd.memset(v_sb[:, :, 0::2, 64:65], 1.0)
            nc.gpsimd.memset(v_sb[:, :, 1::2, 0:1], 1.0)
            nc.gpsimd.memset(v_sb[:, :, 1::2, 1:64], 0.0)

            def emit_bvb():
                ps = psS.tile([P, 2, TG], F32, tag="s", name="bvb_ps")
                nc.tensor.matmul(ps[:, 0, :], ones_sb[0:1, :], bv_row[:, :],
                                 start=True, stop=True)  # bv broadcast to all partitions
                nc.vector.tensor_copy(bv_bc[:], ps[:, 0, :])

            # ---- chain emitters (each allocates one full psS ring tile) ----
            def emit_v(mt):
                ps = psS.tile([P, 2, TG], F32, tag="s", name="ps_v")
                for kc in range(KT):
                    nc.tensor.matmul(
                        ps[:, 0, :],
                        x_sb[kc][:, mt * P:(mt + 1) * P],
                        wv_sb[:, kc, :],
                        start=(kc == 0), stop=(kc == KT - 1),
                    )
                ps3 = ps[:, 0, :].rearrange("p (h d) -> p h d", h=NH)
                bv3 = bv_bc.rearrange("p (h d) -> p h d", h=NH)
                nc.vector.tensor_tensor(v_sb[:, mt, 0::2, 0:64], ps3[:, 0::2, :],
                                        bv3[:, 0::2, :], ALU.add)
                nc.vector.tensor_tensor(v_sb[:, mt, 1::2, 64:128], ps3[:, 1::2, :],
                                        bv3[:, 1::2, :], ALU.add)

            def emit_qk(pr, tg, bias_on_act=False):
                # q chain then k chain; two psS ring allocs (parity-neutral)
                for w_t, b_t, dst in ((wq_sb, bq_c, qT_sb), (wk_sb, bk_c, kT_sb)):
                    ps = psS.tile([P, 2, TG], F32, tag="s", name="ps_qk")
                    for kc in range(KT):
                        nc.tensor.matmul(
                            ps[:, 0, :],
                            w_t[:, kc, pr * P:(pr + 1) * P],
                            x_sb[kc][:, tg * TG:(tg + 1) * TG],
                            start=(kc == 0), stop=(kc == KT - 1),
                        )
                    d = dst[:, pr, tg * TG:(tg + 1) * TG]
                    if bias_on_act:
                        nc.scalar.activation(d, ps[:, 0, :], AF.Identity,
                                             bias=b_t[:, pr:pr + 1])
                    else:
                        nc.vector.tensor_scalar_add(d, ps[:, 0, :], b_t[:, pr:pr + 1])

            def emit_proj(m, tg, move_on_act=False):
                ps = psS.tile([P, 2, TG], F32, tag="s", name="ps_y")
                for p_in in range(KT):
                    nc.tensor.matmul(
                        ps[:, 0, :],
                        wp_sb[:, p_in, m * P:(m + 1) * P],
                        at_sb[:, p_in, tg * TG:(tg + 1) * TG],
                        start=(p_in == 0), stop=(p_in == KT - 1),
                    )
                y_t = wpool.tile([P, TG], F32, tag="y_t", name="y_t")
                if move_on_act:
                    nc.scalar.activation(y_t[:], ps[:, 0, :], AF.Identity,
                                         bias=bp_c[:, m:m + 1])
                else:
                    nc.vector.tensor_scalar_add(y_t[:], ps[:, 0, :], bp_c[:, m:m + 1])
                nc.sync.dma_start(y[m * P:(m + 1) * P, tg * TG:(tg + 1) * TG], y_t[:])

            # ---- Phase A: v chains + first qk chains (DMA-paced) ----
            emit_bvb()
            emit_v(0)
            emit_v(1)
            emit_v(2)
            emit_v(3)
            emit_v(4)
            emit_v(5)
            emit_v(6)
            emit_v(7)
            emit_qk(0, 0, bias_on_act=True)
            emit_qk(0, 1, bias_on_act=True)
            emit_qk(1, 0, bias_on_act=True)

            # ---- Phase B: attention stream (all pairs), with fillers ----
            ALLSTEPS = [(p, tg, kn)
                        for p in range(PAIRS)
                        for tg in range(NTG)
                        for kn in range(4 if tg == 0 else 8)]
            NS = len(ALLSTEPS)  # 48

            a2_of = {}

            def emit_scores(g):
                p, tg, kn = ALLSTEPS[g]
                cs = max(0, P * kn - TG * tg)
                ncols = TG - cs
                s2 = psS.tile([P, 2, TG], F32, tag="s", name="s2")
                nc.tensor.matmul(
                    s2[:, 0, 0:ncols],
                    kT_sb[0:64, p, kn * P:(kn + 1) * P],
                    qT_sb[0:64, p, tg * TG + cs:(tg + 1) * TG],
                    start=True, stop=True,
                )
                nc.tensor.matmul(
                    s2[:, 1, 0:ncols],
                    kT_sb[64:128, p, kn * P:(kn + 1) * P],
                    qT_sb[64:128, p, tg * TG + cs:(tg + 1) * TG],
                    start=True, stop=True,
                )
                a2 = apool.tile([P, 2, TG], BF16, tag="a2", name="a2")
                nc.scalar.activation(a2[:, :, 0:ncols], s2[:, :, 0:ncols], AF.Exp)
                if kn >= 4 * tg:  # block straddles the diagonal
                    nc.gpsimd.affine_select(
                        a2[:, :, 0:P], a2[:, :, 0:P], pattern=[[0, 2], [1, P]],
                        compare_op=ALU.is_ge, fill=0.0,
                        base=0, channel_multiplier=-1,
                    )
                a2_of[g] = a2

            pv_ps = {}

            def emit_pv(g):
                p, tg, kn = ALLSTEPS[g]
                kmax = 4 if tg == 0 else 8
                cs = max(0, P * kn - TG * tg)
                ncols = TG - cs
                if kn == 0:
                    pv_ps["e"] = psO.tile([P, TG], F32, tag="pv_e", name="ps_e")
                    pv_ps["o"] = psO.tile([P, TG], F32, tag="pv_o", name="ps_o")
                a2 = a2_of.pop(g)
                nc.tensor.matmul(
                    pv_ps["e"][0:65, cs:TG],
                    v_sb[:, kn, 2 * p, 0:65],
                    a2[:, 0, 0:ncols],
                    start=(kn == 0), stop=(kn == kmax - 1),
                )
                nc.tensor.matmul(
                    pv_ps["o"][:, cs:TG],
                    v_sb[:, kn, 2 * p + 1, :],
                    a2[:, 1, 0:ncols],
                    start=(kn == 0), stop=(kn == kmax - 1),
                )

            def emit_tail(p, tg):
                ps_e, ps_o = pv_ps["e"], pv_ps["o"]
                rec = wpool.tile([P, TG], BF16, tag="rec", name="rec")
                with nc.allow_low_precision("bf16 softmax denom reciprocal"):
                    nc.vector.reciprocal(rec[64:65, :], ps_e[64:65, :])
                    nc.vector.reciprocal(rec[0:1, :], ps_o[0:1, :])
                bc_ps = psS.tile([P, 2, TG], F32, tag="s", name="bc_ps")
                nc.tensor.matmul(bc_ps[0:64, 0, :], ones_sb[64:65, 0:64],
                                 rec[64:65, :], start=True, stop=True)
                nc.tensor.matmul(bc_ps[64:128, 0, :], ones_sb[0:1, 0:64],
                                 rec[0:1, :], start=True, stop=True)
                # parity filler: keeps the scores ring on its 2-deep cadence
                dummy = psS.tile([P, 2, TG], F32, tag="s", name="dummy")
                nc.tensor.matmul(dummy[0:1, 0, 0:1], ones_sb[0:1, 0:1],
                                 ones_sb[0:1, 0:1], start=True, stop=True)
                bc = wpool.tile([P, TG], F32, tag="bc", name="bc")
                nc.vector.tensor_copy(bc[:], bc_ps[:, 0, :])
                nc.vector.tensor_tensor(
                    at_sb[0:64, p, tg * TG:(tg + 1) * TG],
                    ps_e[0:64, :], bc[0:64, :], ALU.mult)
                nc.vector.tensor_tensor(
                    at_sb[64:128, p, tg * TG:(tg + 1) * TG],
                    ps_o[64:128, :], bc[64:128, :], ALU.mult)

            # filler insertions: slot -> list of thunks (even psS-alloc count each)
            fillers = {
                8: [lambda: emit_qk(1, 1)],
                12: [lambda: emit_qk(2, 0)],
                20: [lambda: emit_qk(2, 1)],
                24: [lambda: emit_qk(3, 0)],
                32: [lambda: emit_qk(3, 1)],
                44: [lambda: emit_proj(0, 0), lambda: emit_proj(1, 0)],
                46: [lambda: emit_proj(2, 0), lambda: emit_proj(3, 0)],
            }

            for g in range(NS + LAG):
                if g < NS:
                    emit_scores(g)
                j = g - LAG
                if 0 <= j:
                    emit_pv(j)
                    pj, tgj, knj = ALLSTEPS[j]
                    if j + 1 == NS or ALLSTEPS[j + 1][1] != tgj:
                        emit_tail(pj, tgj)
                for th in fillers.get(g, ()):
                    th()

            # ---- tail projection (tg=1) ----
            emit_proj(0, 1)
            emit_proj(1, 1)
            emit_proj(2, 1, move_on_act=True)
            emit_proj(3, 1)

    nc.finalize()
    _CACHE["nc"] = nc
    return nc


def _prep_inputs(x, W_qkv, b_qkv, W_proj, b_proj):
    import ml_dtypes
    bf16 = ml_dtypes.bfloat16
    scale = HD ** -0.5
    wq = np.ascontiguousarray(W_qkv[0:C].T * scale).astype(bf16)
    wk = np.ascontiguousarray(W_qkv[C:2 * C].T).astype(bf16)
    wv = np.ascontiguousarray(W_qkv[2 * C:3 * C].T).astype(bf16)
    wp = np.ascontiguousarray(W_proj.T).astype(bf16)
    cpack = np.zeros((P, P + C + 24), dtype=bf16)
    cpack[0, 0:P] = 1
    cpack[64, 0:P] = 1
    cpack[0, P:P + C] = b_qkv[2 * C:3 * C].astype(bf16)
    biases = np.concatenate([np.asarray(b_qkv[0:C] * scale, dtype=np.float32).reshape(KT, P).T,
                             np.asarray(b_qkv[C:2 * C], dtype=np.float32).reshape(KT, P).T,
                             np.asarray(b_proj, dtype=np.float32).reshape(KT, P).T],
                            axis=1)  # [P, 12] f32
    cpack[:, P + C:P + C + 24] = np.ascontiguousarray(biases).view(np.uint16).view(bf16)
    shared = {"wq": wq, "wk": wk, "wv": wv, "wp": wp, "cpack": cpack}
    x_flat = np.ascontiguousarray(x.reshape(B, C, T)).astype(bf16)
    return [dict(shared, x=x_flat[i]) for i in range(B)]


def kernel(x, W_qkv, b_qkv, W_proj, b_proj):
    from concourse import bass_utils
    x = np.asarray(x, dtype=np.float32)
    nc = build_nc()
    in_maps = _prep_inputs(np.asarray(x), np.asarray(W_qkv), np.asarray(b_qkv),
                           np.asarray(W_proj), np.asarray(b_proj))
    res = bass_utils.run_bass_kernel_spmd(nc, in_maps, core_ids=list(range(B)))
    out = np.stack([r["y"] for r in res.results], axis=0)  # (B, C, T)
    return out.reshape(B, C, H, W).astype(np.float32)


# revision 9
# speedup vs baseline: 1.0412x; 1.0412x over previous
"""Causal self-attention 2D kernel for Trainium2 (Bass/Tile), data-parallel over batch.

Problem (hardcoded): x (8, 512, 32, 32) f32, W_qkv (1536, 512), b_qkv (1536,),
W_proj (512, 512), b_proj (512,). seq = 32*32 = 1024 in raster order, 8 heads,
head_dim 64, causal softmax.

Sharding: one batch element per NeuronCore (8 cores). Weights broadcast.

Per-core dataflow (bf16 matmul operands, fp32 accumulation):
  - x[b] stored (C, T) = (512, 1024); weights pre-transposed on host.
  - scores computed TRANSPOSED: s^T[key, query] per head pair, both heads'
    128-key blocks in one 2-bank PSUM tile so a single ACT exp covers them.
  - softmax without max-subtraction; causal masking via gpsimd.affine_select
    on diagonal blocks only; fully masked key blocks skipped everywhere.
  - P.V transposed with v stationary; softmax denominator rides as an extra
    ones column of v (even heads: [v | 1] -> u rows 0:64, denom row 64; odd
    heads: [1 | 0*63 | v] -> u rows 64:128, denom row 0).
  - normalize: DVE reciprocal of denom rows -> gpsimd partition_broadcast
    (no PE matmul, no PSUM) -> DVE multiply into (C, T) attention output.
  - single attention stream over the 4 head pairs, scores double-buffered in
    PSUM (2x [128, 2, 512] tiles) so the PE runs 2 steps ahead of ACT exp;
    PV matmuls lag scores by 2 steps; remaining QKV chains and the tg=0
    projection are injected into specific slots as PE filler; the engine mix
    is balanced so ACT (exp) ~ PE in the attention span.
  - PSUM budget: scores ring 4 banks + two PV accumulator rings 4 banks = 8.
    v/qk/proj chain scratch PSUM is allocated from the scores ring in
    parity-preserving (even-count) groups.
"""

import numpy as np

import concourse.bass as bass
import concourse.mybir as mybir
from concourse import bacc
from concourse.tile import TileContext

F32 = mybir.dt.float32
BF16 = mybir.dt.bfloat16
AF = mybir.ActivationFunctionType
ALU = mybir.AluOpType

B, C, H, W = 8, 512, 32, 32
T = H * W            # 1024 tokens
NH, HD = 8, C // 8   # 8 heads, dim 64
P = 128
KT = C // P          # 4 contraction tiles
TG = 512             # token group (matmul free dim)
NTG = T // TG        # 2
TT = T // P          # 8 token tiles (also key tiles)
PAIRS = NH // 2      # 4 head pairs
LAG = 2

_CACHE = {}


def build_nc():
    if "nc" in _CACHE:
        return _CACHE["nc"]
    nc = bacc.Bacc(None, target_bir_lowering=False)

    x = nc.dram_tensor("x", (C, T), BF16, kind="ExternalInput")
    wq = nc.dram_tensor("wq", (C, C), BF16, kind="ExternalInput")  # (c_in, c_out), pre-scaled
    wk = nc.dram_tensor("wk", (C, C), BF16, kind="ExternalInput")
    wv = nc.dram_tensor("wv", (C, C), BF16, kind="ExternalInput")
    wp = nc.dram_tensor("wp", (C, C), BF16, kind="ExternalInput")
    # packed consts: cols 0:128 ones rows (0,64); row 0 cols 128:640 = bv;
    # cols 640:664 hold 12 f32 bias values (bq|bk|bp) as raw bytes
    cpack = nc.dram_tensor("cpack", (P, P + C + 24), BF16, kind="ExternalInput")
    y = nc.dram_tensor("y", (C, T), F32, kind="ExternalOutput")

    x_r = x.rearrange("(ko p) t -> p ko t", p=P)
    w_r = {n: t.rearrange("(ko p) j -> p ko j", p=P) for n, t in
           [("wq", wq), ("wk", wk), ("wv", wv), ("wp", wp)]}

    with TileContext(nc) as tc:
        with (
            tc.tile_pool(name="const", bufs=1) as cpool,
            tc.tile_pool(name="work", bufs=3) as wpool,
            tc.tile_pool(name="apool", bufs=4) as apool,
            tc.tile_pool(name="psS", bufs=2, space="PSUM") as psS,
            tc.tile_pool(name="psO", bufs=2, space="PSUM") as psO,
        ):
            # ---- persistent SBUF ----
            x_sb = [cpool.tile([P, T], BF16, tag=f"x_sb{kc}", name=f"x_sb{kc}") for kc in range(KT)]
            wq_sb = cpool.tile([P, KT, C], BF16, tag="wq_sb")
            wk_sb = cpool.tile([P, KT, C], BF16, tag="wk_sb")
            wv_sb = cpool.tile([P, KT, C], BF16, tag="wv_sb")
            wp_sb = cpool.tile([P, KT, C], BF16, tag="wp_sb")
            cpack_sb = cpool.tile([P, P + C + 24], BF16, tag="cpack_sb")
            bv_bc = cpool.tile([P, C], F32, tag="bv_bc")
            ones_sb = cpack_sb[:, 0:P]
            bv_row = cpack_sb[0:1, P:P + C]
            qT_sb = cpool.tile([P, PAIRS, T], BF16, tag="qT_sb")   # pair m: heads 2m, 2m+1
            kT_sb = cpool.tile([P, PAIRS, T], BF16, tag="kT_sb")
            v_sb = cpool.tile([P, TT, NH, P], BF16, tag="v_sb")  # [t-part, keytile, head, 128]
            at_sb = cpool.tile([P, PAIRS, T], BF16, tag="at_sb")  # attention out (C, T)

            bias_f32 = cpack_sb[:, P + C:P + C + 24].bitcast(F32)  # [P, 12]
            bq_c = bias_f32[:, 0:KT]
            bk_c = bias_f32[:, KT:2 * KT]
            bp_c = bias_f32[:, 2 * KT:3 * KT]

            # ---- DMAs: v-path first (x + wv), then q/k weights, wp last.
            # Spread issue across sequencers (HWDGE is shared; this overlaps
            # the per-engine DGE setup).
            nc.sync.dma_start(cpack_sb[:, :], cpack[:, :])
            nc.sync.dma_start(wv_sb[:, 0, :], w_r["wv"][:, 0, :])
            nc.sync.dma_start(x_sb[0][:], x_r[:, 0, :])
            nc.sync.dma_start(wv_sb[:, 1:KT, :], w_r["wv"][:, 1:KT, :])
            nc.sync.dma_start(x_sb[1][:], x_r[:, 1, :])
            nc.sync.dma_start(x_sb[2][:], x_r[:, 2, :])
            nc.sync.dma_start(x_sb[3][:], x_r[:, 3, :])
            nc.sync.dma_start(wq_sb[:], w_r["wq"][:, :, :])
            nc.sync.dma_start(wk_sb[:], w_r["wk"][:, :, :])
            nc.sync.dma_start(wp_sb[:], w_r["wp"][:, :, :])

            # ---- gpsimd constants: v_aug columns; bv broadcast ----
            # even heads col 64 = 1; odd heads col 0 = 1, cols 1:64 = 0
            nc.gpsimd.memset(v_sb[:, :, 0::2, 64:65], 1.0)
            nc.gpsimd.memset(v_sb[:, :, 1::2, 0:1], 1.0)
            nc.gpsimd.memset(v_sb[:, :, 1::2, 1:64], 0.0)

            def emit_bvb():
                ps = psS.tile([P, 2, TG], F32, tag="s", name="bvb_ps")
                nc.tensor.matmul(ps[:, 0, :], ones_sb[0:1, :], bv_row[:, :],
                                 start=True, stop=True)  # bv broadcast to all partitions
                nc.vector.tensor_copy(bv_bc[:], ps[:, 0, :])

            # ---- chain emitters (each allocates one full psS ring tile) ----
            def emit_v(mt):
                ps = psS.tile([P, 2, TG], F32, tag="s", name="ps_v")
                for kc in range(KT):
                    nc.tensor.matmul(
                        ps[:, 0, :],
                        x_sb[kc][:, mt * P:(mt + 1) * P],
                        wv_sb[:, kc, :],
                        start=(kc == 0), stop=(kc == KT - 1),
                    )
                ps3 = ps[:, 0, :].rearrange("p (h d) -> p h d", h=NH)
                bv3 = bv_bc.rearrange("p (h d) -> p h d", h=NH)
                nc.vector.tensor_tensor(v_sb[:, mt, 0::2, 0:64], ps3[:, 0::2, :],
                                        bv3[:, 0::2, :], ALU.add)
                nc.vector.tensor_tensor(v_sb[:, mt, 1::2, 64:128], ps3[:, 1::2, :],
                                        bv3[:, 1::2, :], ALU.add)

            def emit_qk(pr, tg, bias_on_act=False):
                # q chain then k chain; two psS ring allocs (parity-neutral)
                for w_t, b_t, dst in ((wq_sb, bq_c, qT_sb), (wk_sb, bk_c, kT_sb)):
                    ps = psS.tile([P, 2, TG], F32, tag="s", name="ps_qk")
                    for kc in range(KT):
                        nc.tensor.matmul(
                            ps[:, 0, :],
                            w_t[:, kc, pr * P:(pr + 1) * P],
                            x_sb[kc][:, tg * TG:(tg + 1) * TG],
                            start=(kc == 0), stop=(kc == KT - 1),
                        )
                    d = dst[:, pr, tg * TG:(tg + 1) * TG]
                    if bias_on_act:
                        nc.scalar.activation(d, ps[:, 0, :], AF.Identity,
                                             bias=b_t[:, pr:pr + 1])
                    else:
                        nc.vector.tensor_scalar_add(d, ps[:, 0, :], b_t[:, pr:pr + 1])

            def emit_proj(m, tg, move_on_act=False):
                ps = psS.tile([P, 2, TG], F32, tag="s", name="ps_y")
                for p_in in range(KT):
                    nc.tensor.matmul(
                        ps[:, 0, :],
                        wp_sb[:, p_in, m * P:(m + 1) * P],
                        at_sb[:, p_in, tg * TG:(tg + 1) * TG],
                        start=(p_in == 0), stop=(p_in == KT - 1),
                    )
                y_t = wpool.tile([P, TG], F32, tag="y_t", name="y_t")
                if move_on_act:
                    nc.scalar.activation(y_t[:], ps[:, 0, :], AF.Identity,
                                         bias=bp_c[:, m:m + 1])
                else:
                    nc.vector.tensor_scalar_add(y_t[:], ps[:, 0, :], bp_c[:, m:m + 1])
                nc.scalar.dma_start(y[m * P:(m + 1) * P, tg * TG:(tg + 1) * TG], y_t[:])

            # ---- Phase A: v chains + first qk chains (DMA-paced) ----
            emit_bvb()
            emit_v(0)
            emit_v(1)
            emit_v(2)
            emit_v(3)
            emit_v(4)
            emit_v(5)
            emit_v(6)
            emit_v(7)
            emit_qk(0, 0, bias_on_act=True)
            emit_qk(0, 1, bias_on_act=True)
            emit_qk(1, 0, bias_on_act=True)

            # ---- Phase B: attention stream (all pairs), with fillers ----
            ALLSTEPS = [(p, tg, kn)
                        for p in range(PAIRS)
                        for tg in range(NTG)
                        for kn in range(4 if tg == 0 else 8)]
            NS = len(ALLSTEPS)  # 48

            a2_of = {}

            def emit_scores(g):
                p, tg, kn = ALLSTEPS[g]
                cs = max(0, P * kn - TG * tg)
                ncols = TG - cs
                s2 = psS.tile([P, 2, TG], F32, tag="s", name="s2")
                nc.tensor.matmul(
                    s2[:, 0, 0:ncols],
                    kT_sb[0:64, p, kn * P:(kn + 1) * P],
                    qT_sb[0:64, p, tg * TG + cs:(tg + 1) * TG],
                    start=True, stop=True,
                )
                nc.tensor.matmul(
                    s2[:, 1, 0:ncols],
                    kT_sb[64:128, p, kn * P:(kn + 1) * P],
                    qT_sb[64:128, p, tg * TG + cs:(tg + 1) * TG],
                    start=True, stop=True,
                )
                a2 = apool.tile([P, 2, TG], BF16, tag="a2", name="a2")
                nc.scalar.activation(a2[:, :, 0:ncols], s2[:, :, 0:ncols], AF.Exp)
                if kn >= 4 * tg:  # block straddles the diagonal
                    nc.gpsimd.affine_select(
                        a2[:, :, 0:P], a2[:, :, 0:P], pattern=[[0, 2], [1, P]],
                        compare_op=ALU.is_ge, fill=0.0,
                        base=0, channel_multiplier=-1,
                    )
                a2_of[g] = a2

            pv_ps = {}

            def emit_pv(g):
                p, tg, kn = ALLSTEPS[g]
                kmax = 4 if tg == 0 else 8
                cs = max(0, P * kn - TG * tg)
                ncols = TG - cs
                if kn == 0:
                    pv_ps["e"] = psO.tile([P, TG], F32, tag="pv_e", name="ps_e")
                    pv_ps["o"] = psO.tile([P, TG], F32, tag="pv_o", name="ps_o")
                a2 = a2_of.pop(g)
                nc.tensor.matmul(
                    pv_ps["e"][0:65, cs:TG],
                    v_sb[:, kn, 2 * p, 0:65],
                    a2[:, 0, 0:ncols],
                    start=(kn == 0), stop=(kn == kmax - 1),
                )
                nc.tensor.matmul(
                    pv_ps["o"][:, cs:TG],
                    v_sb[:, kn, 2 * p + 1, :],
                    a2[:, 1, 0:ncols],
                    start=(kn == 0), stop=(kn == kmax - 1),
                )

            def emit_tail(p, tg):
                ps_e, ps_o = pv_ps["e"], pv_ps["o"]
                rec = wpool.tile([P, TG], BF16, tag="rec", name="rec")
                with nc.allow_low_precision("bf16 softmax denom reciprocal"):
                    nc.vector.reciprocal(rec[64:65, :], ps_e[64:65, :])
                    nc.vector.reciprocal(rec[0:1, :], ps_o[0:1, :])
                bc_ps = psS.tile([P, 2, TG], F32, tag="s", name="bc_ps")
                nc.tensor.matmul(bc_ps[0:64, 0, :], ones_sb[64:65, 0:64],
                                 rec[64:65, :], start=True, stop=True)
                nc.tensor.matmul(bc_ps[64:128, 0, :], ones_sb[0:1, 0:64],
                                 rec[0:1, :], start=True, stop=True)
                # parity filler: keeps the scores ring on its 2-deep cadence
                dummy = psS.tile([P, 2, TG], F32, tag="s", name="dummy")
                nc.tensor.matmul(dummy[0:1, 0, 0:1], ones_sb[0:1, 0:1],
                                 ones_sb[0:1, 0:1], start=True, stop=True)
                bc = wpool.tile([P, TG], F32, tag="bc", name="bc")
                nc.vector.tensor_copy(bc[:], bc_ps[:, 0, :])
                nc.vector.tensor_tensor(
                    at_sb[0:64, p, tg * TG:(tg + 1) * TG],
                    ps_e[0:64, :], bc[0:64, :], ALU.mult)
                nc.vector.tensor_tensor(
                    at_sb[64:128, p, tg * TG:(tg + 1) * TG],
                    ps_o[64:128, :], bc[64:128, :], ALU.mult)

            # filler insertions: slot -> list of thunks (even psS-alloc count each)
            fillers = {
                8: [lambda: emit_qk(1, 1)],
                12: [lambda: emit_qk(2, 0)],
                20: [lambda: emit_qk(2, 1)],
                24: [lambda: emit_qk(3, 0)],
                32: [lambda: emit_qk(3, 1)],
                44: [lambda: emit_proj(0, 0), lambda: emit_proj(1, 0)],
                46: [lambda: emit_proj(2, 0), lambda: emit_proj(3, 0)],
            }

            for g in range(NS + LAG):
                if g < NS:
                    emit_scores(g)
                j = g - LAG
                if 0 <= j:
                    emit_pv(j)
                    pj, tgj, knj = ALLSTEPS[j]
                    if j + 1 == NS or ALLSTEPS[j + 1][1] != tgj:
                        emit_tail(pj, tgj)
                for th in fillers.get(g, ()):
                    th()

            # ---- tail projection (tg=1) ----
            emit_proj(0, 1)
            emit_proj(1, 1)
            emit_proj(2, 1, move_on_act=True)
            emit_proj(3, 1)

    nc.finalize()
    _CACHE["nc"] = nc
    return nc


def _prep_inputs(x, W_qkv, b_qkv, W_proj, b_proj):
    import ml_dtypes
    bf16 = ml_dtypes.bfloat16
    scale = HD ** -0.5
    wq = np.ascontiguousarray(W_qkv[0:C].T * scale).astype(bf16)
    wk = np.ascontiguousarray(W_qkv[C:2 * C].T).astype(bf16)
    wv = np.ascontiguousarray(W_qkv[2 * C:3 * C].T).astype(bf16)
    wp = np.ascontiguousarray(W_proj.T).astype(bf16)
    cpack = np.zeros((P, P + C + 24), dtype=bf16)
    cpack[0, 0:P] = 1
    cpack[64, 0:P] = 1
    cpack[0, P:P + C] = b_qkv[2 * C:3 * C].astype(bf16)
    biases = np.concatenate([np.asarray(b_qkv[0:C] * scale, dtype=np.float32).reshape(KT, P).T,
                             np.asarray(b_qkv[C:2 * C], dtype=np.float32).reshape(KT, P).T,
                             np.asarray(b_proj, dtype=np.float32).reshape(KT, P).T],
                            axis=1)  # [P, 12] f32
    cpack[:, P + C:P + C + 24] = np.ascontiguousarray(biases).view(np.uint16).view(bf16)
    shared = {"wq": wq, "wk": wk, "wv": wv, "wp": wp, "cpack": cpack}
    x_flat = np.ascontiguousarray(x.reshape(B, C, T)).astype(bf16)
    return [dict(shared, x=x_flat[i]) for i in range(B)]


def kernel(x, W_qkv, b_qkv, W_proj, b_proj):
    from concourse import bass_utils
    x = np.asarray(x, dtype=np.float32)
    nc = build_nc()
    in_maps = _prep_inputs(np.asarray(x), np.asarray(W_qkv), np.asarray(b_qkv),
                           np.asarray(W_proj), np.asarray(b_proj))
    res = bass_utils.run_bass_kernel_spmd(nc, in_maps, core_ids=list(range(B)))
    out = np.stack([r["y"] for r in res.results], axis=0)  # (B, C, T)
    return out.reshape(B, C, H, W).astype(np.float32)


# revision 10
# speedup vs baseline: 1.0695x; 1.0272x over previous
"""Causal self-attention 2D kernel for Trainium2 (Bass/Tile), data-parallel over batch.

Problem (hardcoded): x (8, 512, 32, 32) f32, W_qkv (1536, 512), b_qkv (1536,),
W_proj (512, 512), b_proj (512,). seq = 32*32 = 1024 in raster order, 8 heads,
head_dim 64, causal softmax.

Sharding: one batch element per NeuronCore (8 cores). Weights broadcast.

Per-core dataflow (bf16 matmul operands, fp32 accumulation):
  - x[b] stored (C, T) = (512, 1024); weights pre-transposed on host.
  - scores computed TRANSPOSED: s^T[key, query] per head pair, both heads'
    128-key blocks in one 2-bank PSUM tile so a single ACT exp covers them.
  - softmax without max-subtraction; causal masking via gpsimd.affine_select
    on diagonal blocks only; fully masked key blocks skipped everywhere.
  - P.V transposed with v stationary; softmax denominator rides as an extra
    ones column of v (even heads: [v | 1] -> u rows 0:64, denom row 64; odd
    heads: [1 | 0*63 | v] -> u rows 64:128, denom row 0).
  - normalize: DVE reciprocal of denom rows -> gpsimd partition_broadcast
    (no PE matmul, no PSUM) -> DVE multiply into (C, T) attention output.
  - single attention stream over the 4 head pairs, scores double-buffered in
    PSUM (2x [128, 2, 512] tiles) so the PE runs 2 steps ahead of ACT exp;
    PV matmuls lag scores by 2 steps; remaining QKV chains and the tg=0
    projection are injected into specific slots as PE filler; the engine mix
    is balanced so ACT (exp) ~ PE in the attention span.
  - PSUM budget: scores ring 4 banks + two PV accumulator rings 4 banks = 8.
    v/qk/proj chain scratch PSUM is allocated from the scores ring in
    parity-preserving (even-count) groups.
"""

import numpy as np

import concourse.bass as bass
import concourse.mybir as mybir
from concourse import bacc
from concourse.tile import TileContext

F32 = mybir.dt.float32
BF16 = mybir.dt.bfloat16
AF = mybir.ActivationFunctionType
ALU = mybir.AluOpType

B, C, H, W = 8, 512, 32, 32
T = H * W            # 1024 tokens
NH, HD = 8, C // 8   # 8 heads, dim 64
P = 128
KT = C // P          # 4 contraction tiles
TG = 512             # token group (matmul free dim)
NTG = T // TG        # 2
TT = T // P          # 8 token tiles (also key tiles)
PAIRS = NH // 2      # 4 head pairs
LAG = 2

_CACHE = {}


def build_nc():
    if "nc" in _CACHE:
        return _CACHE["nc"]
    nc = bacc.Bacc(None, target_bir_lowering=False)

    x = nc.dram_tensor("x", (C, T), BF16, kind="ExternalInput")
    wq = nc.dram_tensor("wq", (C, C), BF16, kind="ExternalInput")  # (c_in, c_out), pre-scaled
    wk = nc.dram_tensor("wk", (C, C), BF16, kind="ExternalInput")
    wv = nc.dram_tensor("wv", (C, C), BF16, kind="ExternalInput")
    wp = nc.dram_tensor("wp", (C, C), BF16, kind="ExternalInput")
    # packed consts: cols 0:128 ones rows (0,64); row 0 cols 128:640 = bv;
    # cols 640:664 hold 12 f32 bias values (bq|bk|bp) as raw bytes
    cpack = nc.dram_tensor("cpack", (P, P + C + 24), BF16, kind="ExternalInput")
    y = nc.dram_tensor("y", (C, T), F32, kind="ExternalOutput")

    x_r = x.rearrange("(ko p) t -> p ko t", p=P)
    w_r = {n: t.rearrange("(ko p) j -> p ko j", p=P) for n, t in
           [("wq", wq), ("wk", wk), ("wv", wv), ("wp", wp)]}

    with TileContext(nc) as tc:
        with (
            tc.tile_pool(name="const", bufs=1) as cpool,
            tc.tile_pool(name="work", bufs=3) as wpool,
            tc.tile_pool(name="apool", bufs=4) as apool,
            tc.tile_pool(name="psS", bufs=2, space="PSUM") as psS,
            tc.tile_pool(name="psO", bufs=2, space="PSUM") as psO,
        ):
            # ---- persistent SBUF ----
            x_sb = [cpool.tile([P, T], BF16, tag=f"x_sb{kc}", name=f"x_sb{kc}") for kc in range(KT)]
            wq_sb = cpool.tile([P, KT, C], BF16, tag="wq_sb")
            wk_sb = cpool.tile([P, KT, C], BF16, tag="wk_sb")
            wv_sb = cpool.tile([P, KT, C], BF16, tag="wv_sb")
            wp_sb = cpool.tile([P, KT, C], BF16, tag="wp_sb")
            cpack_sb = cpool.tile([P, P + C + 24], BF16, tag="cpack_sb")
            bv_bc = cpool.tile([P, C], F32, tag="bv_bc")
            ones_sb = cpack_sb[:, 0:P]
            bv_row = cpack_sb[0:1, P:P + C]
            qT_sb = cpool.tile([P, PAIRS, T], BF16, tag="qT_sb")   # pair m: heads 2m, 2m+1
            kT_sb = cpool.tile([P, PAIRS, T], BF16, tag="kT_sb")
            v_sb = cpool.tile([P, TT, NH, P], BF16, tag="v_sb")  # [t-part, keytile, head, 128]
            at_sb = cpool.tile([P, PAIRS, T], BF16, tag="at_sb")  # attention out (C, T)

            bias_f32 = cpack_sb[:, P + C:P + C + 24].bitcast(F32)  # [P, 12]
            bq_c = bias_f32[:, 0:KT]
            bk_c = bias_f32[:, KT:2 * KT]
            bp_c = bias_f32[:, 2 * KT:3 * KT]

            # ---- DMAs: v-path first (x + wv), then q/k weights, wp last.
            # Spread issue across sequencers (HWDGE is shared; this overlaps
            # the per-engine DGE setup).
            nc.sync.dma_start(cpack_sb[:, :], cpack[:, :])
            nc.sync.dma_start(wv_sb[:, 0, :], w_r["wv"][:, 0, :])
            nc.sync.dma_start(x_sb[0][:], x_r[:, 0, :])
            nc.sync.dma_start(wv_sb[:, 1:KT, :], w_r["wv"][:, 1:KT, :])
            nc.sync.dma_start(x_sb[1][:], x_r[:, 1, :])
            nc.sync.dma_start(x_sb[2][:], x_r[:, 2, :])
            nc.sync.dma_start(x_sb[3][:], x_r[:, 3, :])
            nc.sync.dma_start(wq_sb[:], w_r["wq"][:, :, :])
            nc.sync.dma_start(wk_sb[:], w_r["wk"][:, :, :])
            nc.sync.dma_start(wp_sb[:], w_r["wp"][:, :, :])

            # ---- gpsimd constants: v_aug columns; bv broadcast ----
            # even heads col 64 = 1; odd heads col 0 = 1, cols 1:64 = 0
            nc.gpsimd.memset(v_sb[:, :, 0::2, 64:65], 1.0)
            nc.gpsimd.memset(v_sb[:, :, 1::2, 0:1], 1.0)
            nc.gpsimd.memset(v_sb[:, :, 1::2, 1:64], 0.0)

            def emit_bvb():
                ps = psS.tile([P, 2, TG], F32, tag="s", name="bvb_ps")
                nc.tensor.matmul(ps[:, 0, :], ones_sb[0:1, :], bv_row[:, :],
                                 start=True, stop=True)  # bv broadcast to all partitions
                nc.vector.tensor_copy(bv_bc[:], ps[:, 0, :])

            # ---- chain emitters (each allocates one full psS ring tile) ----
            def emit_v(mt):
                ps = psS.tile([P, 2, TG], F32, tag="s", name="ps_v")
                for kc in range(KT):
                    nc.tensor.matmul(
                        ps[:, 0, :],
                        x_sb[kc][:, mt * P:(mt + 1) * P],
                        wv_sb[:, kc, :],
                        start=(kc == 0), stop=(kc == KT - 1),
                    )
                ps3 = ps[:, 0, :].rearrange("p (h d) -> p h d", h=NH)
                bv3 = bv_bc.rearrange("p (h d) -> p h d", h=NH)
                nc.vector.tensor_tensor(v_sb[:, mt, 0::2, 0:64], ps3[:, 0::2, :],
                                        bv3[:, 0::2, :], ALU.add)
                nc.vector.tensor_tensor(v_sb[:, mt, 1::2, 64:128], ps3[:, 1::2, :],
                                        bv3[:, 1::2, :], ALU.add)

            def emit_qk(pr, tg, bias_on_act=False):
                # q chain then k chain; two psS ring allocs (parity-neutral)
                for w_t, b_t, dst in ((wq_sb, bq_c, qT_sb), (wk_sb, bk_c, kT_sb)):
                    ps = psS.tile([P, 2, TG], F32, tag="s", name="ps_qk")
                    for kc in range(KT):
                        nc.tensor.matmul(
                            ps[:, 0, :],
                            w_t[:, kc, pr * P:(pr + 1) * P],
                            x_sb[kc][:, tg * TG:(tg + 1) * TG],
                            start=(kc == 0), stop=(kc == KT - 1),
                        )
                    d = dst[:, pr, tg * TG:(tg + 1) * TG]
                    if bias_on_act:
                        nc.scalar.activation(d, ps[:, 0, :], AF.Identity,
                                             bias=b_t[:, pr:pr + 1])
                    else:
                        nc.vector.tensor_scalar_add(d, ps[:, 0, :], b_t[:, pr:pr + 1])

            def emit_proj(m, tg, move_on_act=False):
                ps = psS.tile([P, 2, TG], F32, tag="s", name="ps_y")
                for p_in in range(KT):
                    nc.tensor.matmul(
                        ps[:, 0, :],
                        wp_sb[:, p_in, m * P:(m + 1) * P],
                        at_sb[:, p_in, tg * TG:(tg + 1) * TG],
                        start=(p_in == 0), stop=(p_in == KT - 1),
                    )
                y_t = wpool.tile([P, TG], F32, tag="y_t", name="y_t")
                if move_on_act:
                    nc.scalar.activation(y_t[:], ps[:, 0, :], AF.Identity,
                                         bias=bp_c[:, m:m + 1])
                else:
                    nc.vector.tensor_scalar_add(y_t[:], ps[:, 0, :], bp_c[:, m:m + 1])
                nc.scalar.dma_start(y[m * P:(m + 1) * P, tg * TG:(tg + 1) * TG], y_t[:])

            # ---- Phase A: v chains + first qk chains (DMA-paced) ----
            emit_bvb()
            emit_v(0)
            emit_v(1)
            emit_v(2)
            emit_v(3)
            emit_v(4)
            emit_v(5)
            emit_v(6)
            emit_v(7)
            emit_qk(0, 0, bias_on_act=True)
            emit_qk(0, 1, bias_on_act=True)
            emit_qk(1, 0, bias_on_act=True)

            # ---- Phase B: attention stream (all pairs), with fillers ----
            ALLSTEPS = [(p, tg, kn)
                        for p in range(PAIRS)
                        for tg in range(NTG)
                        for kn in range(4 if tg == 0 else 8)]
            NS = len(ALLSTEPS)  # 48

            a2_of = {}

            def emit_scores(g):
                p, tg, kn = ALLSTEPS[g]
                cs = max(0, P * kn - TG * tg)
                ncols = TG - cs
                s2 = psS.tile([P, 2, TG], F32, tag="s", name="s2")
                nc.tensor.matmul(
                    s2[:, 0, 0:ncols],
                    kT_sb[0:64, p, kn * P:(kn + 1) * P],
                    qT_sb[0:64, p, tg * TG + cs:(tg + 1) * TG],
                    start=True, stop=True,
                )
                nc.tensor.matmul(
                    s2[:, 1, 0:ncols],
                    kT_sb[64:128, p, kn * P:(kn + 1) * P],
                    qT_sb[64:128, p, tg * TG + cs:(tg + 1) * TG],
                    start=True, stop=True,
                )
                a2 = apool.tile([P, 2, TG], BF16, tag="a2", name="a2")
                nc.scalar.activation(a2[:, :, 0:ncols], s2[:, :, 0:ncols], AF.Exp)
                if kn >= 4 * tg:  # block straddles the diagonal
                    nc.gpsimd.affine_select(
                        a2[:, :, 0:P], a2[:, :, 0:P], pattern=[[0, 2], [1, P]],
                        compare_op=ALU.is_ge, fill=0.0,
                        base=0, channel_multiplier=-1,
                    )
                a2_of[g] = a2

            pv_ps = {}

            def emit_pv(g):
                p, tg, kn = ALLSTEPS[g]
                kmax = 4 if tg == 0 else 8
                cs = max(0, P * kn - TG * tg)
                ncols = TG - cs
                if kn == 0:
                    pv_ps["e"] = psO.tile([P, TG], F32, tag="pv_e", name="ps_e")
                    pv_ps["o"] = psO.tile([P, TG], F32, tag="pv_o", name="ps_o")
                a2 = a2_of.pop(g)
                nc.tensor.matmul(
                    pv_ps["e"][0:65, cs:TG],
                    v_sb[:, kn, 2 * p, 0:65],
                    a2[:, 0, 0:ncols],
                    start=(kn == 0), stop=(kn == kmax - 1),
                )
                nc.tensor.matmul(
                    pv_ps["o"][:, cs:TG],
                    v_sb[:, kn, 2 * p + 1, :],
                    a2[:, 1, 0:ncols],
                    start=(kn == 0), stop=(kn == kmax - 1),
                )

            def emit_tail_a(p, tg):
                # reciprocals only (DVE); the PE-side broadcast is deferred two
                # slots so the PE never waits on these in its in-order stream
                ps_e, ps_o = pv_ps["e"], pv_ps["o"]
                rec = wpool.tile([P, TG], BF16, tag="rec", name="rec")
                with nc.allow_low_precision("bf16 softmax denom reciprocal"):
                    nc.vector.reciprocal(rec[64:65, :], ps_e[64:65, :])
                    nc.vector.reciprocal(rec[0:1, :], ps_o[0:1, :])
                return ps_e, ps_o, rec

            def emit_tail_b(p, tg, saved):
                ps_e, ps_o, rec = saved
                bc_ps = psS.tile([P, 2, TG], F32, tag="s", name="bc_ps")
                nc.tensor.matmul(bc_ps[0:64, 0, :], ones_sb[64:65, 0:64],
                                 rec[64:65, :], start=True, stop=True)
                nc.tensor.matmul(bc_ps[64:128, 0, :], ones_sb[0:1, 0:64],
                                 rec[0:1, :], start=True, stop=True)
                # parity filler: keeps the scores ring on its 2-deep cadence
                dummy = psS.tile([P, 2, TG], F32, tag="s", name="dummy")
                nc.tensor.matmul(dummy[0:1, 0, 0:1], ones_sb[0:1, 0:1],
                                 ones_sb[0:1, 0:1], start=True, stop=True)
                bc = wpool.tile([P, TG], F32, tag="bc", name="bc")
                nc.vector.tensor_copy(bc[:], bc_ps[:, 0, :])
                nc.vector.tensor_tensor(
                    at_sb[0:64, p, tg * TG:(tg + 1) * TG],
                    ps_e[0:64, :], bc[0:64, :], ALU.mult)
                nc.vector.tensor_tensor(
                    at_sb[64:128, p, tg * TG:(tg + 1) * TG],
                    ps_o[64:128, :], bc[64:128, :], ALU.mult)

            # partial proj chains for (m, tg=1), accumulated in the pv-ring
            # buffers freed after pair 3's tg0 tail; finished at the end
            part_ps = {}

            def emit_proj_part(m, tag, p_hi):
                ps = psO.tile([P, TG], F32, tag=tag, name=f"pp{m}")
                for p_in in range(p_hi):
                    nc.tensor.matmul(
                        ps[:],
                        wp_sb[:, p_in, m * P:(m + 1) * P],
                        at_sb[:, p_in, TG:2 * TG],
                        start=(p_in == 0), stop=False,
                    )
                part_ps[m] = ps

            def emit_proj_fin(m, move_on_act=False):
                ps = part_ps.pop(m)
                nc.tensor.matmul(
                    ps[:],
                    wp_sb[:, 3, m * P:(m + 1) * P],
                    at_sb[:, 3, TG:2 * TG],
                    start=False, stop=True,
                )
                y_t = wpool.tile([P, TG], F32, tag="y_t", name="y_t")
                if move_on_act:
                    nc.scalar.activation(y_t[:], ps[:], AF.Identity,
                                         bias=bp_c[:, m:m + 1])
                else:
                    nc.vector.tensor_scalar_add(y_t[:], ps[:], bp_c[:, m:m + 1])
                nc.scalar.dma_start(y[m * P:(m + 1) * P, TG:2 * TG], y_t[:])

            # filler insertions: slot -> list of thunks (even psS-alloc count each)
            fillers = {
                8: [lambda: emit_qk(1, 1)],
                12: [lambda: emit_qk(2, 0)],
                20: [lambda: emit_qk(2, 1)],
                24: [lambda: emit_qk(3, 0)],
                32: [lambda: emit_qk(3, 1)],
                45: [lambda: emit_proj(0, 0), lambda: emit_proj(1, 0)],
                46: [lambda: emit_proj_part(0, "pv_e", 3)],
                47: [lambda: emit_proj(2, 0), lambda: emit_proj(3, 0),
                     lambda: emit_proj_part(1, "pv_o", 3)],
            }

            pending_tail_b = {}
            for g in range(NS + LAG + 2):
                if g < NS:
                    emit_scores(g)
                j = g - LAG
                if 0 <= j < NS:
                    emit_pv(j)
                    pj, tgj, knj = ALLSTEPS[j]
                    if j + 1 == NS or ALLSTEPS[j + 1][1] != tgj:
                        pending_tail_b[g + 2] = (pj, tgj, emit_tail_a(pj, tgj))
                if g in pending_tail_b:
                    pj, tgj, saved = pending_tail_b.pop(g)
                    emit_tail_b(pj, tgj, saved)
                for th in fillers.get(g, ()):
                    th()

            # ---- tail projection (tg=1) ----
            emit_proj_fin(0)
            emit_proj_fin(1, move_on_act=True)
            emit_proj(2, 1, move_on_act=True)
            emit_proj(3, 1)

    nc.finalize()
    _CACHE["nc"] = nc
    return nc


def _prep_inputs(x, W_qkv, b_qkv, W_proj, b_proj):
    import ml_dtypes
    bf16 = ml_dtypes.bfloat16
    scale = HD ** -0.5
    wq = np.ascontiguousarray(W_qkv[0:C].T * scale).astype(bf16)
    wk = np.ascontiguousarray(W_qkv[C:2 * C].T).astype(bf16)
    wv = np.ascontiguousarray(W_qkv[2 * C:3 * C].T).astype(bf16)
    wp = np.ascontiguousarray(W_proj.T).astype(bf16)
    cpack = np.zeros((P, P + C + 24), dtype=bf16)
    cpack[0, 0:P] = 1
    cpack[64, 0:P] = 1
    cpack[0, P:P + C] = b_qkv[2 * C:3 * C].astype(bf16)
    biases = np.concatenate([np.asarray(b_qkv[0:C] * scale, dtype=np.float32).reshape(KT, P).T,
                             np.asarray(b_qkv[C:2 * C], dtype=np.float32).reshape(KT, P).T,
                             np.asarray(b_proj, dtype=np.float32).reshape(KT, P).T],
                            axis=1)  # [P, 12] f32
    cpack[:, P + C:P + C + 24] = np.ascontiguousarray(biases).view(np.uint16).view(bf16)
    shared = {"wq": wq, "wk": wk, "wv": wv, "wp": wp, "cpack": cpack}
    x_flat = np.ascontiguousarray(x.reshape(B, C, T)).astype(bf16)
    return [dict(shared, x=x_flat[i]) for i in range(B)]


def kernel(x, W_qkv, b_qkv, W_proj, b_proj):
    from concourse import bass_utils
    x = np.asarray(x, dtype=np.float32)
    nc = build_nc()
    in_maps = _prep_inputs(np.asarray(x), np.asarray(W_qkv), np.asarray(b_qkv),
                           np.asarray(W_proj), np.asarray(b_proj))
    res = bass_utils.run_bass_kernel_spmd(nc, in_maps, core_ids=list(range(B)))
    out = np.stack([r["y"] for r in res.results], axis=0)  # (B, C, T)
    return out.reshape(B, C, H, W).astype(np.float32)
